# revision 18
# baseline (speedup 1.0000x reference)
"""Trainium2 Bass kernel for nn_C3SNN_ModelT: CNN feature extractor + LIF SNN.

v3 redesign:
- conv: direct DRAM->SBUF im2col (no DRAM staging round-trip) via a
  repermuted t27 partition order p = 12*(3kx+ky) + 4c + s, 9 large DMAs
  per chunk; fused XY-reduce maxpool epilogues (DVE) with i-pair max on
  GpSimd and relu+bias on Act; merged compact+rep96 replication DMA.
- x-lo plane uploaded as fp8*512 (scale cancelled by a w1h/512
  stationary), halving the t27-lo DMA traffic.
- SNN: ve-recurrence encoder on DVE (stt ops are DVE-only on this ISA;
  GpSimd elementwise is ~18x slower -- DMA/memset only), spike
  thresholds via DVE tensor_scalar + Act sign/relu, full-width 128-col
  fc matmuls (64-col splits stall on LDWEIGHTS: ld time = M_cols/1.2GHz
  must hide under the moving stream).

Data parallel over 8 cores (128 samples each).
"""
import os
import sys
sys.path.insert(0, "/opt/trn_rl_repo")

import ml_dtypes
import numpy as np
import concourse.bass as bass
import concourse.mybir as mybir
import concourse.tile as tile
from concourse import bacc
from concourse.bass_utils import run_bass_kernel_spmd

CONV3 = True   # conv wh@xl correction pass (feat precision needs it)
FC2P = True    # fc1/fc2 fp16 lo passes (fc1 single-pass fails the err gate)
FCDR = False   # fp8 DoubleRow fc lo: no win (LDWEIGHTS-bound at 128 cols)
FC_SCALE = 512.0                                  # exact for z/512 in fp8

F32 = mybir.dt.float32
F16 = mybir.dt.float16
F8 = mybir.dt.float8e4
DR = mybir.MatmulPerfMode.DoubleRow
MAX = mybir.AluOpType.max
MULT = mybir.AluOpType.mult
ADD = mybir.AluOpType.add
SUB = mybir.AluOpType.subtract
MOD = mybir.AluOpType.mod
IS_GT = mybir.AluOpType.is_gt
IS_LE = mybir.AluOpType.is_le
IS_LT = mybir.AluOpType.is_lt
RELU = mybir.ActivationFunctionType.Relu
SIGN = mybir.ActivationFunctionType.Sign
LN = mybir.ActivationFunctionType.Ln
COPY = mybir.ActivationFunctionType.Copy
AXX = mybir.AxisListType.X
AXY = mybir.AxisListType.XY

N_CORES = 8
BPC = 128          # batch per core
BB = 16            # conv batch chunk
NCHUNK = BPC // BB
SEQ = 32
C_LOG = -9.491221581028618   # 1/ln(0.9)

LAST_EXEC_NS = None
_CACHE = {}


def build_nc(debug_outputs=False, do_conv=True, seq=SEQ):
    nc = bacc.Bacc(None, target_bir_lowering=False, debug=False)

    # ---- DRAM I/O ----
    # x staged host-side as [ci, c, s, g, h, w] so each (kx, ky) shift is
    # one 3-dim DMA: partitions (c s) merged, free (g, hw-window)
    xh = nc.dram_tensor("xh", [NCHUNK, 3, 4, 4, 34, 34], F16,
                        kind="ExternalInput")
    xl = (nc.dram_tensor("xl", [NCHUNK, 3, 4, 4, 34, 34], F8,
                         kind="ExternalInput") if CONV3 else None)
    w1h = nc.dram_tensor("w1h", [108, 128], F16, kind="ExternalInput")
    w1h5 = nc.dram_tensor("w1h5", [108, 128], F16, kind="ExternalInput")
    w1l = nc.dram_tensor("w1l", [108, 128], F16, kind="ExternalInput")
    w2h = nc.dram_tensor("w2h", [3, 96, 64], F16, kind="ExternalInput")
    w2l = nc.dram_tensor("w2l", [3, 96, 64], F16, kind="ExternalInput")
    w3ah = nc.dram_tensor("w3ah", [3, 128, 64], F16, kind="ExternalInput")
    w3al = nc.dram_tensor("w3al", [3, 128, 64], F16, kind="ExternalInput")
    w3bh = nc.dram_tensor("w3bh", [3, 64, 64], F16, kind="ExternalInput")
    w3bl = nc.dram_tensor("w3bl", [3, 64, 64], F16, kind="ExternalInput")
    cb1 = nc.dram_tensor("cb1", [128, 1], F32, kind="ExternalInput")
    cb2 = nc.dram_tensor("cb2", [64, 1], F32, kind="ExternalInput")
    cb3 = nc.dram_tensor("cb3", [64, 1], F32, kind="ExternalInput")  # 0.4*b3
    fc1h = nc.dram_tensor("fc1h", [128, 8 * 4 * 128], F16, kind="ExternalInput")
    fc2h = nc.dram_tensor("fc2h", [128, 4 * 2 * 128], F16, kind="ExternalInput")
    fc1l = (nc.dram_tensor("fc1l", [128, 8 * 4 * 128], F16,
                           kind="ExternalInput") if FC2P else None)
    fc2l = (nc.dram_tensor("fc2l", [128, 4 * 2 * 128], F16,
                           kind="ExternalInput") if FC2P else None)
    fc1d = (nc.dram_tensor("fc1d", [128, 4 * 2 * 4 * 128], F8,
                           kind="ExternalInput") if FCDR else None)
    fc2d = (nc.dram_tensor("fc2d", [128, 2 * 2 * 2 * 128], F8,
                           kind="ExternalInput") if FCDR else None)
    libeta = nc.dram_tensor("libeta", [128, 2 * SEQ * 10], F16,
                            kind="ExternalInput")
    id10 = nc.dram_tensor("id10", [10, 10], F32, kind="ExternalInput")
    out = nc.dram_tensor("out", [BPC, 10], F32, kind="ExternalOutput")
    dbg = {}
    if debug_outputs:
        dbg["featT"] = nc.dram_tensor("dbg_featT", [128, 8, 128], F32,
                                      kind="ExternalOutput")

    with tile.TileContext(nc) as tc:
        with (
            tc.tile_pool(name="wpool", bufs=1) as wpool,
            tc.tile_pool(name="state", bufs=1) as state,
        ):
            # weights to SBUF
            w1hs = wpool.tile([108, 128], F16)
            w1h5s = wpool.tile([108, 128], F16)
            w1ls = wpool.tile([108, 128], F16)
            w2hs = wpool.tile([96, 3, 64], F16)
            w2ls = wpool.tile([96, 3, 64], F16)
            w3ahs = wpool.tile([128, 3, 64], F16)
            w3als = wpool.tile([128, 3, 64], F16)
            w3bhs = wpool.tile([64, 3, 64], F16)
            w3bls = wpool.tile([64, 3, 64], F16)
            cb1s = wpool.tile([128, 1], F32)
            cb2s = wpool.tile([64, 1], F32)
            cb3s = wpool.tile([64, 1], F32)
            nth4 = wpool.tile([128, 1], F32)
            nth1 = wpool.tile([128, 1], F32)
            id10s = wpool.tile([10, 10], F32)
            nc.gpsimd.memset(nth4[:], -4.0)
            nc.gpsimd.memset(nth1[:], -1.0)
            for dst_t, src_t in [(w1hs, w1h), (w1h5s, w1h5),
                                 (w1ls, w1l), (cb1s, cb1),
                                 (cb2s, cb2), (cb3s, cb3), (id10s, id10)]:
                nc.sync.dma_start(dst_t[:], src_t[:])
            for dst_t, src_t in [(w2hs, w2h), (w2ls, w2l), (w3ahs, w3ah),
                                 (w3als, w3al), (w3bhs, w3bh), (w3bls, w3bl)]:
                nc.sync.dma_start(dst_t[:],
                                  src_t[:].rearrange("k p n -> p k n"))

            # featT: scaled features (0.1*feat), [p=(jl,ch64), k(8), b(128)]
            featT = state.tile([128, 8, 128], F32)

            if do_conv:
                build_conv(nc, tc, xh, xl, featT, w1hs, w1h5s, w1ls,
                           w2hs, w2ls, w3ahs, w3als, w3bhs, w3bls,
                           cb1s, cb2s, cb3s)
            else:
                nc.vector.memset(featT[:], 0.0)

            if debug_outputs:
                nc.sync.dma_start(dbg["featT"][:], featT[:])

            build_snn(nc, tc, state, featT, fc1h, fc1l, fc2h, fc2l,
                      fc1d, fc2d, libeta, id10s, nth4, nth1, out, seq)

    nc.compile()
    return nc


def build_conv(nc, tc, xh, xl, featT, w1hs, w1h5s, w1ls, w2hs, w2ls,
               w3ahs, w3als, w3bhs, w3bls, cb1s, cb2s, cb3s):
    # t27 partition order: p = 12*q + 4*c + s with q = 3*kx + ky.
    # Sample mapping: b = 16*ci + 4*g + s.
    xv = xh[:].rearrange("ci c s g h w -> ci (c s) g (h w)")
    xvl = (xl[:].rearrange("ci c s g h w -> ci (c s) g (h w)")
           if CONV3 else None)
    # Act queue is reserved for compute epilogues; DMAs go on sync/gpsimd
    dmaq = [nc.sync, nc.gpsimd]

    with (
        tc.tile_pool(name="conv_in", bufs=2) as conv_in,
        tc.tile_pool(name="conv_rep", bufs=1) as conv_rep,
        tc.tile_pool(name="conv_sc", bufs=3) as csc,
        tc.tile_pool(name="pl1", bufs=3, space="PSUM") as pl1,
        tc.tile_pool(name="pl2", bufs=3, space="PSUM") as pl2,
        tc.tile_pool(name="pl23", bufs=2, space="PSUM") as pl23,
    ):
        nrep = 2 if CONV3 else 1
        # ky-replicated tiles (borders must stay zero; memset once per buf)
        rep96s = [[conv_rep.tile([96, 16, 16, 18], F16, tag=f"r96{v}{i}",
                                 name=f"r96{v}{i}") for i in range(2)]
                  for v in range(nrep)]
        repas = [[conv_rep.tile([128, 8, 10, 16], F16, tag=f"rpa{v}{i}",
                                name=f"rpa{v}{i}") for i in range(2)]
                 for v in range(nrep)]
        repbs = [[conv_rep.tile([64, 8, 10, 16], F16, tag=f"rpb{v}{i}",
                                name=f"rpb{v}{i}") for i in range(2)]
                 for v in range(nrep)]
        for v in range(nrep):
            for i in range(2):
                nc.vector.memset(rep96s[v][i][:], 0.0)
                nc.vector.memset(repas[v][i][:], 0.0)
                nc.vector.memset(repbs[v][i][:], 0.0)
        l2ds = [[conv_rep.tile([128, 4, 16, 18], F16, tag=f"l2d{v}{i}",
                               name=f"l2d{v}{i}") for i in range(2)]
                for v in range(nrep)]
        l3ds = [[conv_rep.tile([64, 8, 10, 16], F16, tag=f"l3d{v}{i}",
                               name=f"l3d{v}{i}") for i in range(2)]
                for v in range(nrep)]
        for v in range(nrep):
            for i in range(2):
                t_ = l2ds[v][i]
                nc.vector.memset(t_[:, :, :, 0:1], 0.0)
                nc.vector.memset(t_[:, :, :, 17:18], 0.0)
                t3 = l3ds[v][i]
                nc.vector.memset(t3[:, :, 0:1, :], 0.0)
                nc.vector.memset(t3[:, :, 9:10, :], 0.0)

        def load_t27(ci):
            t27h = conv_in.tile([108, 4, 1088], F16, tag="t27h", name="t27h")
            tiles = [(t27h, xv)]
            if CONV3:
                t27l = conv_in.tile([108, 4, 1088], F8, tag="t27l",
                                    name="t27l")
                tiles.append((t27l, xvl))
            n = 0
            for dst, src in tiles:
                for kx in range(3):
                    for ky in range(3):
                        q = 3 * kx + ky
                        s0 = ky * 34 + kx
                        L = min(1088, 1156 - s0)
                        dmaq[n % 2].dma_start(
                            dst[12 * q:12 * q + 12, :, 0:L],
                            src[ci, :, :, s0:s0 + L])
                        n += 1
            return tiles

        def phase_l1(ci, t27s):
            bi = ci % 2
            vh = t27s[0][0][:].rearrange("p g (i j) -> p g i j", j=34)
            vl = (t27s[1][0][:].rearrange("p g (i j) -> p g i j", j=34)
                  if CONV3 else None)
            l1passes = [(w1hs, vh), (w1ls, vh)]
            if CONV3:
                # x-lo uploaded as fp8*512; stationary pre-divided by 512
                l1passes.insert(1, (w1h5s, vl))
            for g in range(4):
                for half in range(2):
                    ps = pl1.tile([128, 16, 32], F32, tag="ps1", name="ps1")
                    i0 = 16 * half
                    for pi, (wst, vt) in enumerate(l1passes):
                        nc.tensor.matmul(
                            ps[:], wst[:], vt[0:108, g, i0:i0 + 16, 0:32],
                            start=(pi == 0), stop=(pi == len(l1passes) - 1))
                    r = ps[:].rearrange(
                        "p (i2 ii) (j2 jj) -> p i2 j2 ii jj", ii=2, jj=2)
                    oh = l2ds[0][bi][:, g, 8 * half:8 * half + 8, 1:17]
                    if not CONV3:
                        p1 = csc.tile([128, 8, 16], F16, tag="c1a",
                                      name="c1a")
                        nc.vector.tensor_reduce(p1[:], r, AXY, MAX)
                        nc.scalar.activation(oh, p1[:], RELU, bias=cb1s[:])
                    else:
                        p1 = csc.tile([128, 8, 16], F32, tag="c1a",
                                      name="c1a")
                        nc.vector.tensor_reduce(p1[:], r, AXY, MAX)
                        p3 = csc.tile([128, 8, 16], F32, tag="c1c",
                                      name="c1c")
                        nc.scalar.activation(p3[:], p1[:], RELU, bias=cb1s[:])
                        ol = l2ds[1][bi][:, g, 8 * half:8 * half + 8, 1:17]
                        nc.scalar.activation(oh, p3[:], COPY)
                        nc.vector.tensor_tensor(ol, p3[:], oh, SUB)

        def rep96_merge(ci):
            bi = ci % 2
            n = 0
            for v in range(nrep):
                l2d, rep = l2ds[v][bi], rep96s[v][bi]
                for ky in range(3):
                    for s in range(4):
                        src_p = l2d[32 * s:32 * s + 32, :, :, :]
                        if ky == 0:
                            d = rep[0:32, s:16:4, 1:16, :]
                            sr = src_p[:, :, 0:15, :]
                        elif ky == 1:
                            d = rep[32:64, s:16:4, 0:16, :]
                            sr = src_p
                        else:
                            d = rep[64:96, s:16:4, 0:15, :]
                            sr = src_p[:, :, 1:16, :]
                        dmaq[n % 2].dma_start(d, sr)
                        n += 1

        def l2_compute(ci):
            bi = ci % 2
            r96h = rep96s[0][bi]
            passes = [(w2hs, r96h), (w2ls, r96h)]
            if CONV3:
                passes.insert(1, (w2hs, rep96s[1][bi]))
            nmm = 3 * len(passes)
            for pg in range(8):
                ps = pl2.tile([64, 2, 16, 16], F32, tag="ps2", name="ps2")
                i_mm = 0
                for kx in range(3):
                    for wst, rep in passes:
                        nc.tensor.matmul(
                            ps[:], wst[:, kx, :],
                            rep[0:96, 2 * pg:2 * pg + 2, :, kx:kx + 16],
                            start=(i_mm == 0), stop=(i_mm == nmm - 1))
                        i_mm += 1
                rv = ps[:].rearrange("p s i (j2 jj) -> p s i j2 jj", jj=2)
                oh = l3ds[0][bi][:, :, 1:9, 2 * pg:2 * pg + 2].rearrange(
                    "p i j s -> p s i j")
                if not CONV3:
                    q1 = csc.tile([64, 2, 16, 8], F16, tag="c2a", name="c2a")
                    nc.vector.tensor_reduce(q1[:], rv, AXX, MAX)
                    q14 = q1[:].rearrange("p s (i2 ii) j -> p s i2 ii j",
                                          ii=2)
                    p2 = csc.tile([64, 2, 8, 8], F16, tag="c2b", name="c2b")
                    nc.vector.tensor_tensor(p2[:], q14[:, :, :, 0, :],
                                            q14[:, :, :, 1, :], MAX)
                    nc.scalar.activation(oh, p2[:], RELU, bias=cb2s[:])
                else:
                    q1 = csc.tile([64, 2, 16, 8], F32, tag="c2a", name="c2a")
                    nc.vector.tensor_reduce(q1[:], rv, AXX, MAX)
                    q14 = q1[:].rearrange("p s (i2 ii) j -> p s i2 ii j",
                                          ii=2)
                    p2 = csc.tile([64, 2, 8, 8], F32, tag="c2b", name="c2b")
                    nc.vector.tensor_tensor(p2[:], q14[:, :, :, 0, :],
                                            q14[:, :, :, 1, :], MAX)
                    p3 = csc.tile([64, 2, 8, 8], F32, tag="c2c", name="c2c")
                    nc.scalar.activation(p3[:], p2[:], RELU, bias=cb2s[:])
                    ol = l3ds[1][bi][:, :, 1:9, 2 * pg:2 * pg + 2].rearrange(
                        "p i j s -> p s i j")
                    nc.scalar.activation(oh, p3[:], COPY)
                    nc.vector.tensor_tensor(ol, p3[:], oh, SUB)

        def rep3_merge(ci):
            bi = ci % 2
            n = 1
            for v in range(nrep):
                src, ra, rb = l3ds[v][bi], repas[v][bi], repbs[v][bi]
                dmaq[n % 2].dma_start(ra[0:64, 1:8, :, :], src[:, 0:7, :, :])
                dmaq[(n + 1) % 2].dma_start(ra[64:128, 0:8, :, :], src[:])
                dmaq[n % 2].dma_start(rb[0:64, 0:7, :, :],
                                      src[:, 1:8, :, :])
                n += 3

        def l3_compute(ci):
            bi = ci % 2
            rah, rbh = repas[0][bi], repbs[0][bi]
            apasses = [(w3ahs, rah), (w3als, rah)]
            bpasses = [(w3bhs, rbh), (w3bls, rbh)]
            if CONV3:
                apasses.insert(1, (w3ahs, repas[1][bi]))
                bpasses.insert(1, (w3bhs, repbs[1][bi]))
            ntot = 3 * (len(apasses) + len(bpasses))
            b0 = 16 * ci
            for ih in range(2):
                i0 = 4 * ih
                ps = pl23.tile([64, 4, 8, 16], F32, tag="ps3", name="ps3")
                i_mm = 0
                for kx in range(3):
                    for (wa, ra) in apasses:
                        nc.tensor.matmul(
                            ps[:], wa[:, kx, :],
                            ra[0:128, i0:i0 + 4, kx:kx + 8, :],
                            start=(i_mm == 0), stop=False)
                        i_mm += 1
                    for (wb, rb) in bpasses:
                        nc.tensor.matmul(
                            ps[:], wb[:, kx, :],
                            rb[0:64, i0:i0 + 4, kx:kx + 8, :],
                            start=False, stop=(i_mm == ntot - 1))
                        i_mm += 1
                rv = ps[:].rearrange("p i (j2 jj) s -> p i j2 s jj", jj=2)
                q3 = csc.tile([64, 4, 4, 16], F32, tag="c3a", name="c3a")
                nc.vector.tensor_reduce(q3[:], rv, AXX, MAX)
                q34 = q3[:].rearrange("p (i2 ii) j s -> p i2 ii j s", ii=2)
                p2 = csc.tile([64, 2, 4, 16], F32, tag="c3b", name="c3b")
                nc.vector.tensor_tensor(p2[:], q34[:, :, 0, :, :],
                                        q34[:, :, 1, :, :], MAX)
                # relu(0.4*x + 0.4*b3) = 0.1 * feat
                p3 = csc.tile([64, 2, 4, 16], F32, tag="c3c", name="c3c")
                nc.scalar.activation(p3[:], p2[:], RELU, bias=cb3s[:],
                                     scale=0.4)
                # featT[64*jl + ch, b, k=4ih+2i2+jh] = p3[ch, i2, (jh,jl), s]
                for jl in range(2):
                    sv = p3[:, :, jl:4:2, :].rearrange(
                        "p a jh s -> p (a jh) s")
                    nc.sync.dma_start(
                        featT[64 * jl:64 * jl + 64, 4 * ih:4 * ih + 4,
                              b0:b0 + 16],
                        sv)

        # software pipeline: rep-merges feed computes one iteration later
        t27s_ = {0: load_t27(0)}
        for k in range(NCHUNK + 3):
            if 0 <= k - 1 < NCHUNK:
                rep96_merge(k - 1)
            if 0 <= k - 3 < NCHUNK:
                rep3_merge(k - 3)
            if k + 1 < NCHUNK:
                t27s_[k + 1] = load_t27(k + 1)
            if 0 <= k - 2 < NCHUNK:
                l2_compute(k - 2)
            if 0 <= k - 3 < NCHUNK:
                l3_compute(k - 3)
            if k < NCHUNK:
                phase_l1(k, t27s_.pop(k))


def build_snn(nc, tc, state, featT, fc1h, fc1l, fc2h, fc2l,
              fc1d, fc2d, libeta, id10s, nth4, nth1, out, seq):
    with (
        tc.tile_pool(name="snn_w", bufs=1) as swp,
        tc.tile_pool(name="snn_sc", bufs=2) as ssc,
        tc.tile_pool(name="pc1", bufs=2, space="PSUM") as pc1,
        tc.tile_pool(name="pli", bufs=1, space="PSUM") as pli,
    ):
        fc1hs = swp.tile([128, 8 * 4 * 128], F16)
        fc2hs = swp.tile([128, 4 * 2 * 128], F16)
        libs = swp.tile([128, 2, SEQ, 10], F16)
        nc.sync.dma_start(fc1hs[:], fc1h[:])
        nc.gpsimd.dma_start(fc2hs[:], fc2h[:])
        nc.scalar.dma_start(libs[:], libeta[:].rearrange(
            "p (k t n) -> p k t n", k=2, t=SEQ))
        fc1h4 = fc1hs.rearrange("p (k m n) -> p k m n", k=8, m=4)
        fc2h4 = fc2hs.rearrange("p (k m n) -> p k m n", k=4, m=2)
        if FC2P:
            fc1ls = swp.tile([128, 8 * 4 * 128], F16)
            fc2ls = swp.tile([128, 4 * 2 * 128], F16)
            nc.sync.dma_start(fc1ls[:], fc1l[:])
            nc.scalar.dma_start(fc2ls[:], fc2l[:])
            fc1l4 = fc1ls.rearrange("p (k m n) -> p k m n", k=8, m=4)
            fc2l4 = fc2ls.rearrange("p (k m n) -> p k m n", k=4, m=2)
        if FCDR:
            fc1ds = swp.tile([128, 4 * 2 * 4 * 128], F8)
            fc2ds = swp.tile([128, 2 * 2 * 2 * 128], F8)
            nc.sync.dma_start(fc1ds[:], fc1d[:])
            nc.scalar.dma_start(fc2ds[:], fc2d[:])
            fc1d4 = fc1ds.rearrange("p (kd j m n) -> p kd j m n", kd=4, j=2,
                                    m=4)
            fc2d4 = fc2ds.rearrange("p (kd j m n) -> p kd j m n", kd=2, j=2,
                                    m=2)

        ve = state.tile([128, 8, 128], F32)
        vsc = state.tile([128, 6, 128], F32)
        ic = state.tile([128, 6, 128], F32)
        z16 = state.tile([128, 8, 128], F16)
        sc16 = state.tile([128, 6, 128], F16)
        for t_ in (ve, vsc, ic):
            nc.vector.memset(t_[:], 0.0)

        psl = pli.tile([10, 128], F32)

        for t in range(seq):
            # encoder: q = 0.9*ve + 0.1*feat; z = q > 1; ve = q*(q<=1)
            q = ssc.tile([128, 8, 128], F32, tag="q", name="q")
            nc.vector.scalar_tensor_tensor(q[:], ve[:], 0.9, featT[:],
                                           MULT, ADD)
            nc.vector.tensor_scalar(z16[:], q[:], 1.0, None, IS_GT)
            nc.vector.scalar_tensor_tensor(ve[:], q[:], 1.0, q[:],
                                           IS_LE, MULT)

            # LIF dynamics (th=4.0, states x10); vd uses OLD ic
            vd = ssc.tile([128, 6, 128], F32, tag="vd", name="vd")
            nc.vector.scalar_tensor_tensor(vd[:], vsc[:], 0.9, ic[:],
                                           MULT, ADD)
            # spikes on Act: sc16 = relu(sign(vd-4))
            sg = ssc.tile([128, 6, 128], F16, tag="sg", name="sg")
            nc.scalar.activation(sg[:], vd[:], SIGN, bias=nth4[:])
            nc.scalar.activation(sc16[:], sg[:], RELU)
            nc.vector.scalar_tensor_tensor(vsc[:], vd[:], 4.0, vd[:],
                                           IS_LE, MULT)

            # fc1 -> psc[:, 0:4]; fc2 -> psc[:, 4:6]
            psc = pc1.tile([128, 6, 128], F32, tag="psc", name="psc")
            for m in range(4):
                for k in range(8):
                    nc.tensor.matmul(
                        psc[:, m, :], fc1h4[:, k, m, :], z16[:, k, :],
                        start=(k == 0), stop=(k == 7 and not FC2P))
                if FC2P:
                    for k in range(8):
                        nc.tensor.matmul(
                            psc[:, m, :], fc1l4[:, k, m, :], z16[:, k, :],
                            start=False, stop=(k == 7))
            for m in range(2):
                for k in range(4):
                    nc.tensor.matmul(
                        psc[:, 4 + m, :], fc2h4[:, k, m, :], sc16[:, k, :],
                        start=(k == 0), stop=(k == 3 and not FC2P))
                if FC2P:
                    for k in range(4):
                        nc.tensor.matmul(
                            psc[:, 4 + m, :], fc2l4[:, k, m, :],
                            sc16[:, k, :],
                            start=False, stop=(k == 3))
            # LILinear fold: psl += (beta_t * li_w_k) @ s2_t
            for k in range(2):
                nc.tensor.matmul(
                    psl[:], libs[:, k, t, :], sc16[:, 4 + k, :],
                    start=(t == 0 and k == 0),
                    stop=(t == seq - 1 and k == 1),
                    skip_group_check=True)
            # i' = 0.8*i + cur (both layers; after fc1+fc2 land)
            nc.vector.scalar_tensor_tensor(ic[:], ic[:], 0.8, psc[:],
                                           MULT, ADD)

        vlT = state.tile([10, 128], F32)
        nc.vector.tensor_copy(vlT[:], psl[:])
        with tc.tile_pool(name="pout", bufs=1, space="PSUM") as pout:
            pso = pout.tile([128, 10], F32)
            nc.tensor.transpose(pso[:], vlT[:], id10s[:])
            ot = state.tile([128, 10], F32)
            nc.vector.tensor_copy(ot[:], pso[:])
            nc.sync.dma_start(out[:], ot[:])


def _split16(a):
    hi = a.astype(np.float16)
    lo = (a - hi.astype(np.float32)).astype(np.float16)
    return hi, lo


def prep_weights(w1, b1, w2, b2, w3, b3, fc1_w, fc1_b, fc2_w, fc2_b, li_w):
    d = {}
    # w1 rows permuted to match t27 partition order p = 12*q + 4*c + s
    w1t = w1.transpose(3, 2, 1, 0).reshape(27, 32).astype(np.float32)
    w1bd = np.zeros((108, 128), np.float32)
    for q in range(9):          # q = 3*kx + ky; w1t row = 9*kx + 3*ky + c
        kx, ky = divmod(q, 3)
        for c in range(3):
            for s in range(4):
                w1bd[12 * q + 4 * c + s, 32 * s:32 * s + 32] = \
                    w1t[9 * kx + 3 * ky + c]
    d["w1h"], d["w1l"] = _split16(w1bd)
    d["w1h5"] = (d["w1h"].astype(np.float32) / 512.0).astype(np.float16)
    w2t = np.ascontiguousarray(
        w2.transpose(3, 2, 1, 0).reshape(3, 96, 64).astype(np.float32))
    d["w2h"], d["w2l"] = _split16(w2t)
    w3t = w3.transpose(3, 2, 1, 0).reshape(3, 192, 64).astype(np.float32)
    d["w3ah"], d["w3al"] = _split16(np.ascontiguousarray(w3t[:, :128]))
    d["w3bh"], d["w3bl"] = _split16(np.ascontiguousarray(w3t[:, 128:]))
    d["cb1"] = np.tile(b1.astype(np.float32), 4).reshape(128, 1)
    d["cb2"] = b2.astype(np.float32).reshape(64, 1)
    d["cb3"] = (0.4 * b3.astype(np.float32)).reshape(64, 1)
    # fc1: input features f = q*64 + ch; featT partition p = 64*(q&1) + ch
    perm = np.zeros(1024, np.int64)
    for k in range(8):
        for p in range(128):
            q = 2 * k + (p >> 6)
            perm[k * 128 + p] = (p & 63) * 16 + q
    fc1t = fc1_w.T[perm].astype(np.float32)            # [1024, 512]
    a1 = fc1t.reshape(8, 128, 4, 128).transpose(1, 0, 2, 3)  # [p, k, m, n]
    d["fc1h"] = a1.reshape(128, -1).astype(np.float16)
    fc2t = fc2_w.T.astype(np.float32)                  # [512, 256]
    a2 = fc2t.reshape(4, 128, 2, 128).transpose(1, 0, 2, 3)
    d["fc2h"] = a2.reshape(128, -1).astype(np.float16)
    if FC2P:
        d["fc1l"] = (a1 - d["fc1h"].reshape(a1.shape).astype(np.float32)
                     ).reshape(128, -1).astype(np.float16)
        d["fc2l"] = (a2 - d["fc2h"].reshape(a2.shape).astype(np.float32)
                     ).reshape(128, -1).astype(np.float16)
    if FCDR:
        # lo residual, scaled by FC_SCALE, paired k-tiles for DoubleRow:
        # [p, kd, j, m, n] with k = 2*kd + j
        lo1 = (a1 - d["fc1h"].reshape(a1.shape).astype(np.float32)) * FC_SCALE
        lo1 = lo1.reshape(128, 4, 2, 4, 128)
        d["fc1d"] = lo1.reshape(128, -1).astype(ml_dtypes.float8_e4m3fn)
        lo2 = (a2 - d["fc2h"].reshape(a2.shape).astype(np.float32)) * FC_SCALE
        lo2 = lo2.reshape(128, 2, 2, 2, 128)
        d["fc2d"] = lo2.reshape(128, -1).astype(ml_dtypes.float8_e4m3fn)
    # LILinear fold coefficients: vl_T = sum_t beta_t * (li_w @ s2_t)
    T = SEQ
    beta = []
    for tau in range(1, T + 1):
        b = 0.9 ** (T - tau)
        for t in range(tau + 1, T + 1):
            b += 0.9 ** (T - t) * 0.8 ** (t - tau)
        beta.append(0.1 * b)
    lit = li_w.T.astype(np.float32)                    # [256, 10]
    lib = np.zeros((128, 2, T, 10), np.float32)
    for k in range(2):
        for t in range(T):
            lib[:, k, t, :] = beta[t] * lit[128 * k:128 * k + 128]
    d["libeta"] = lib.reshape(128, -1).astype(np.float16)
    d["id10"] = np.eye(10, dtype=np.float32)
    assert not np.any(fc1_b) and not np.any(fc2_b), \
        "nonzero fc biases not implemented"
    return d


def kernel(x, w1, b1, w2, b2, w3, b3, fc1_w, fc1_b, fc2_w, fc2_b, li_w,
           trace=False, debug_outputs=False):
    global LAST_EXEC_NS
    key = ("nc", debug_outputs)
    if key not in _CACHE:
        _CACHE[key] = build_nc(debug_outputs=debug_outputs)
    nc = _CACHE[key]
    wd = prep_weights(w1, b1, w2, b2, w3, b3, fc1_w, fc1_b, fc2_w, fc2_b, li_w)
    in_maps = []
    for c in range(N_CORES):
        m = dict(wd)
        xs = x[c * BPC:(c + 1) * BPC].astype(np.float32)
        xp = np.pad(xs, ((0, 0), (0, 0), (1, 1), (1, 1)))
        # [b=16ci+4g+s, c, h, w] -> [ci, c, s, g, h, w]
        xp = np.ascontiguousarray(
            xp.reshape(NCHUNK, 4, 4, 3, 34, 34).transpose(0, 3, 2, 1, 4, 5))
        xph = xp.astype(np.float16)
        m["xh"] = xph
        if CONV3:
            m["xl"] = ((xp - xph.astype(np.float32)) * 512.0).astype(
                ml_dtypes.float8_e4m3fn)
        in_maps.append(m)
    res = run_bass_kernel_spmd(nc, in_maps, list(range(N_CORES)), trace=trace)
    LAST_EXEC_NS = res.exec_time_ns
    if debug_outputs:
        kernel.last_results = res.results
    return np.concatenate([res.results[c]["out"] for c in range(N_CORES)], 0)
